# revision 52
# baseline (speedup 1.0000x reference)
"""GraphConv GNN (4-layer + mean-pool + head) on 8 Trainium2 NeuronCores.

Strategy:
  - Host relabels nodes into 8 shards x W_CNT windows x 128 slots, balancing
    per-window in-degree (snake deal by degree + 2D refinement).
  - Each core owns its dst shard. Per layer: build a node-major bf16 table
    (width 128), AllGather it to a full DRAM table, dma_gather per-edge
    source rows into SBUF (4 SWDGE queues in parallel), scatter-add via
    bf16 one-hot matmuls (PE) into PSUM (feature-major agg), fold in the
    root term, bias+ReLU on ScalarE.
  - Layer 1 gathers straight from a host-prepared bf16 x table: no prepass,
    no first-layer collective.
  - Aggregation is pushed to width 128 everywhere: L1 aggregates raw x
    (project after), L2 aggregates p2 = h1 @ w2_rel (project before),
    L3 aggregates h2 (project after), L4 aggregates [p4 | p4] duplicated.
  - Graph mean-pool partials via one-hot matmul; final head on host.
"""

import sys

if "/opt/trn_rl_repo" not in sys.path:
    sys.path.insert(0, "/opt/trn_rl_repo")

import numpy as np
import ml_dtypes

BF16 = ml_dtypes.bfloat16


def _ensure_ntff_hook_module():
    """bass_utils imports antenv.axon_hooks for trace=True under axon;
    some containers lack it. Provide a functional stand-in."""
    try:
        import antenv.axon_hooks  # noqa: F401
        return
    except ImportError:
        pass
    import types

    mod = types.ModuleType("antenv.axon_hooks")
    mod._hook = None

    def set_axon_ntff_profile_hook(hook):
        mod._hook = hook

    def get_axon_ntff_profile_hook():
        if mod._hook is None:
            try:
                from trn_agent_boot.trn_boot import _ntff_profile_via_ctypes

                mod._hook = _ntff_profile_via_ctypes(
                    "/opt/axon/libaxon_pjrt.so"
                )
            except Exception:
                return None
        return mod._hook

    mod.set_axon_ntff_profile_hook = set_axon_ntff_profile_hook
    mod.get_axon_ntff_profile_hook = get_axon_ntff_profile_hook
    try:
        import antenv

        antenv.axon_hooks = mod
    except ImportError:
        pass
    sys.modules["antenv.axon_hooks"] = mod


_ensure_ntff_hook_module()

CORES = 8
WIN_P = 128  # nodes per window == SBUF partitions
HID = 64
WG = 4  # windows per gather group

_PROGRAM_CACHE: dict = {}


# --------------------------------------------------------------------------
# Host-side planning
# --------------------------------------------------------------------------
def _plan(x, src, dst, batch, n_graphs):
    n_nodes = x.shape[0]
    # windows per core, forced even so the table splits at a window
    # boundary into equal halves (collective A/B split)
    w_cnt = 2 * (-(-n_nodes // (CORES * WIN_P * 2)))
    hsplit = w_cnt // 2
    hrows = hsplit * WIN_P
    n_win_tot = CORES * w_cnt
    shard = w_cnt * WIN_P
    npad = CORES * shard
    half = npad // 2
    assert half <= 32768

    indeg = np.bincount(dst, minlength=n_nodes)
    order = np.argsort(-indeg, kind="stable")
    # snake-deal nodes (desc degree) across all windows; slot = deal round
    idxs = np.arange(n_nodes)
    rounds = idxs // n_win_tot
    pos = idxs % n_win_tot
    wsel = np.where(rounds % 2 == 0, pos, n_win_tot - 1 - pos)
    gw = np.empty(n_nodes, np.int64)
    slot = np.empty(n_nodes, np.int64)
    gw[order] = wsel
    slot[order] = rounds
    assert slot.max() < WIN_P

    # refine: 2D greedy vector balancing of (lo,hi) in-degree per window,
    # so the per-(window,half) edge quota can round down to 1024
    for _ in range(2):
        src_hi = ((gw % w_cnt)[src] >= hsplit).astype(np.int64)
        lo_deg = np.zeros(n_nodes, np.int64)
        hi_deg = np.zeros(n_nodes, np.int64)
        np.add.at(lo_deg, dst[src_hi == 0], 1)
        np.add.at(hi_deg, dst[src_hi == 1], 1)
        deg2 = lo_deg + hi_deg
        order2 = np.argsort(-deg2, kind="stable")
        llo = np.zeros(n_win_tot, np.float64)
        lhi = np.zeros(n_win_tot, np.float64)
        ncount = np.zeros(n_win_tot, np.int64)
        gw_new = np.empty(n_nodes, np.int64)
        slot_new = np.empty(n_nodes, np.int64)
        for n in order2:
            score = (llo + lo_deg[n]) ** 2 + (lhi + hi_deg[n]) ** 2
            score[ncount >= WIN_P] = np.inf
            w = int(np.argmin(score))
            gw_new[n] = w
            slot_new[n] = ncount[w]
            ncount[w] += 1
            llo[w] += lo_deg[n]
            lhi[w] += hi_deg[n]
        gw, slot = gw_new, slot_new

    # targeted swap refinement: push every (window,half) cell under 1024
    src_hi = ((gw % w_cnt)[src] >= hsplit).astype(np.int64)
    lo_deg = np.zeros(n_nodes, np.int64)
    hi_deg = np.zeros(n_nodes, np.int64)
    np.add.at(lo_deg, dst[src_hi == 0], 1)
    np.add.at(hi_deg, dst[src_hi == 1], 1)
    llo = np.zeros(n_win_tot, np.int64)
    lhi = np.zeros(n_win_tot, np.int64)
    np.add.at(llo, gw, lo_deg)
    np.add.at(lhi, gw, hi_deg)
    win_nodes = [list(np.where(gw == w)[0]) for w in range(n_win_tot)]
    target = 1024
    for _ in range(4000):
        loads = np.stack([llo, lhi])
        h_star, w_star = np.unravel_index(np.argmax(loads), loads.shape)
        if loads[h_star, w_star] <= target:
            break
        hd = hi_deg if h_star else lo_deg
        od = lo_deg if h_star else hi_deg
        oth = llo if h_star else lhi
        # node in w_star with max h-degree; receiver window minimizing
        # post-swap max of its cells
        cand = win_nodes[w_star]
        a = cand[int(np.argmax(hd[cand]))]
        combined = (loads[h_star] + hd[a]) + 0.25 * (oth + od[a])
        combined[w_star] = np.inf
        # keep node-half labels invariant: only swap within the same
        # half (by window index within a core)
        wins = np.arange(n_win_tot)
        same_side = (wins % w_cnt >= hsplit) == (w_star % w_cnt >= hsplit)
        combined[~same_side] = np.inf
        w_to = int(np.argmin(combined))
        bcand = win_nodes[w_to]
        b = bcand[int(np.argmin(hd[bcand]))]
        # swap a <-> b
        win_nodes[w_star].remove(a)
        win_nodes[w_to].remove(b)
        win_nodes[w_star].append(b)
        win_nodes[w_to].append(a)
        for h, dgs in ((0, lo_deg), (1, hi_deg)):
            ld = llo if h == 0 else lhi
            ld[w_star] += dgs[b] - dgs[a]
            ld[w_to] += dgs[a] - dgs[b]
        gw[a], gw[b] = w_to, w_star
    # recompute slots from final window membership
    for w in range(n_win_tot):
        for i, n in enumerate(win_nodes[w]):
            slot[n] = i

    core_of = gw // w_cnt
    w_of = gw % w_cnt
    lid = w_of * WIN_P + slot  # local node id within shard
    # table row id: lo half is core-major over windows [0, hsplit),
    # hi half core-major over [hsplit, w_cnt) — matches the split
    # AllGather layout (collective A fills rows [0, half))
    pid = np.where(
        lid < hrows,
        core_of * hrows + lid,
        half + core_of * (shard - hrows) + (lid - hrows),
    )

    ecore = core_of[dst]
    ew = w_of[dst]
    ehalf = (pid[src] >= half).astype(np.int64)
    edloc = slot[dst]

    cell = (ecore * w_cnt + ew) * 2 + ehalf
    ccounts = np.bincount(cell, minlength=n_win_tot * 2)
    q = max(int(-(-ccounts.max() // WIN_P) * WIN_P), WIN_P)

    groups = [list(range(g, min(g + WG, w_cnt))) for g in range(0, w_cnt, WG)]
    s_tot = w_cnt * 2 * q

    # slot offset of each (window, half) in the per-core edge array
    off = np.zeros((w_cnt, 2), np.int64)
    o = 0
    for g in groups:
        for h in (0, 1):
            for w in g:
                off[w, h] = o
                o += q
    assert o == s_tot

    idx16 = np.zeros((CORES, s_tot), np.int16)
    dloc = np.full((CORES, s_tot), -1, np.int64)

    eorder = np.argsort(cell, kind="stable")
    sorted_cell = cell[eorder]
    cell_starts = np.zeros(n_win_tot * 2 + 1, np.int64)
    np.cumsum(np.bincount(cell, minlength=n_win_tot * 2), out=cell_starts[1:])
    rank = np.arange(len(eorder)) - cell_starts[sorted_cell]
    p = off[ew[eorder], ehalf[eorder]] + rank
    ec = ecore[eorder]
    idx16[ec, p] = (pid[src] - ehalf * half)[eorder].astype(np.int16)
    dloc[ec, p] = edloc[eorder]

    # wrapped index layout: [C, 128, S/16]; 16-partition pattern replicated x8
    idxw = idx16.reshape(CORES, s_tot // 16, 16).transpose(0, 2, 1)
    idx_tile = np.ascontiguousarray(np.tile(idxw, (1, 8, 1)))

    dloc_tile = np.ascontiguousarray(
        dloc.astype(np.float32)
        .reshape(CORES, s_tot // WIN_P, WIN_P)
        .transpose(0, 2, 1)
    ).astype(BF16)

    f_in = x.shape[1]
    xnode = np.zeros((npad, f_in), BF16)
    xnode[pid] = x.astype(BF16)
    x_t = np.zeros((CORES, f_in, shard), np.float32)
    x_t[core_of, :, lid] = x.astype(np.float32)
    x_t = x_t.astype(BF16)
    bpool = np.zeros((CORES, shard, n_graphs), np.float32)
    bpool[core_of, lid, batch] = 1.0
    bpool = bpool.astype(BF16)

    return dict(
        w_cnt=w_cnt, q=q, shard=shard, npad=npad, half=half, groups=groups,
        s_tot=s_tot, idx_tile=idx_tile, dloc_tile=dloc_tile, x_t=x_t,
        xnode=xnode, bpool=bpool, n_graphs=n_graphs,
    )


# --------------------------------------------------------------------------
# Bass program
# --------------------------------------------------------------------------
def _build_program(w_cnt, q, n_graphs, f_in=128):
    import concourse.bacc as bacc
    import concourse.mybir as mybir
    from concourse import tile

    dt = mybir.dt
    f32 = dt.float32
    bf16 = dt.bfloat16
    alu = mybir.AluOpType
    act = mybir.ActivationFunctionType

    shard = w_cnt * WIN_P
    npad = CORES * shard
    half = npad // 2
    hrows = (w_cnt // 2) * WIN_P
    s_tot = w_cnt * 2 * q
    qc = q // WIN_P  # chunks per (window, half)
    groups = [list(range(g, min(g + WG, w_cnt))) for g in range(0, w_cnt, WG)]
    gmaxc = WG * qc  # max chunk cols in one gather buffer
    rg = [list(range(CORES))]

    nc = bacc.Bacc(
        "TRN2", target_bir_lowering=False, debug=False,
        enable_asserts=False, num_devices=CORES,
        num_swdge_queues=4,
    )

    def din(name, shape, dtyp=bf16):
        return nc.dram_tensor(name, shape, dtyp, kind="ExternalInput").ap()

    xnode = din("xnode", [npad, 128])
    xT = din("xT", [f_in, shard])
    idx = din("idx", [128, s_tot // 16], dt.int16)
    dlc = din("dloc", [128, s_tot // WIN_P])
    bp = din("bpool", [shard, n_graphs])
    iota = din("iota", [128, qc * 128])
    ident = din("ident", [128, 128])
    w1r = din("w1_rel", [128, 64])
    w1o = din("w1_root", [128, 64])
    b1 = din("b1", [64, 1], f32)
    w2r = din("w2_rel", [64, 128])
    w2o = din("w2_root", [64, 128])
    b2 = din("b2", [128, 1], f32)
    w3r = din("w3_rel", [128, 192])
    w3o = din("w3_root", [128, 192])
    b3a = din("b3a", [128, 1], f32)
    b3b = din("b3b", [64, 1], f32)
    w4ra = din("w4_rel_a", [128, 64])
    w4rb = din("w4_rel_b", [64, 64])
    w4oa = din("w4_root_a", [128, 128])  # cols 64: zero-padded
    w4ob = din("w4_root_b", [64, 128])
    b4 = din("b4", [64, 1], f32)
    pooled = nc.dram_tensor(
        "pooled", [n_graphs, HID], f32, kind="ExternalOutput"
    ).ap()

    with tile.TileContext(nc) as tc:
        with (
            tc.tile_pool(name="const", bufs=1) as cp,
            tc.tile_pool(name="hbuf", bufs=1) as hp,
            tc.tile_pool(name="gather", bufs=4) as gp,
            tc.tile_pool(name="dmat", bufs=4) as dp,
            tc.tile_pool(name="stage", bufs=3) as sp,
            tc.tile_pool(name="aggs", bufs=3) as ap_,
            tc.tile_pool(name="dram", bufs=1, space="DRAM") as dram,
            tc.tile_pool(name="ps_agg", bufs=3, space="PSUM") as pagg,
            tc.tile_pool(name="ps_aux", bufs=2, space="PSUM") as paux,
            tc.tile_pool(name="ps_h", bufs=2, space="PSUM") as ph,
            tc.tile_pool(name="ps_pool", bufs=1, space="PSUM") as ppl,
        ):
            # ---- persistent SBUF loads -----------------------------------
            s_idx = cp.tile([128, s_tot // 16], dt.int16)
            nc.sync.dma_start(s_idx[:], idx[:])
            s_dlc = cp.tile([128, s_tot // WIN_P], bf16)
            nc.sync.dma_start(s_dlc[:], dlc[:])
            s_iota = cp.tile([128, qc * 128], bf16)
            nc.sync.dma_start(s_iota[:], iota[:])
            s_id = cp.tile([128, 128], bf16)
            nc.sync.dma_start(s_id[:], ident[:])

            def load(apx, shape, dtyp=bf16):
                t = cp.tile(shape, dtyp, name=f"w_{apx.tensor.name}")
                nc.sync.dma_start(t[:], apx[:])
                return t

            s_w1r = load(w1r, [128, 64])
            s_w1o = load(w1o, [128, 64])
            s_b1 = load(b1, [64, 1], f32)
            s_w2r = load(w2r, [64, 128])
            s_w2o = load(w2o, [64, 128])
            s_b2 = load(b2, [128, 1], f32)
            s_w3r = load(w3r, [128, 192])
            s_w3o = load(w3o, [128, 192])
            s_b3a = load(b3a, [128, 1], f32)
            s_b3b = load(b3b, [64, 1], f32)
            s_w4ra = load(w4ra, [128, 64])
            s_w4rb = load(w4rb, [64, 64])
            s_w4oa = load(w4oa, [128, 128])
            s_w4ob = load(w4ob, [64, 128])
            s_b4 = load(b4, [64, 1], f32)

            s_xT = hp.tile([f_in, shard], bf16, name="s_xT")
            nc.sync.dma_start(s_xT[:], xT[:])

            h1T = hp.tile([64, shard], bf16, name="h1T")
            h2T = hp.tile([128, shard], bf16, name="h2T")
            h3aT = hp.tile([128, shard], bf16, name="h3aT")
            h3bT = hp.tile([64, shard], bf16, name="h3bT")
            h4T = hp.tile([64, shard], bf16, name="h4T")

            # ---- DRAM bounce + shared tables (layers 2..4) ---------------
            # one Shared tensor per half: Shared tensors allow only a
            # single writer instruction, and we AllGather halves separately
            tbl_in = [None]
            tbl_a = [None]
            tbl_b = [None]
            for i in range(1, 4):
                ti = dram.tile([shard, 128], bf16, name=f"tblin{i}")
                ta = dram.tile([half, 128], bf16, name=f"tbl{i}a",
                               addr_space="Shared")
                tb = dram.tile([npad - half, 128], bf16, name=f"tbl{i}b",
                               addr_space="Shared")
                tbl_in.append(ti)
                tbl_a.append(ta)
                tbl_b.append(tb)

            qctr = [0]

            # ---- helper: one aggregation pass ----------------------------
            def agg_pass(layer, t_lo, t_hi, tail, tail_accumulates):
                """tail(w, psum_agg) consumes the [128,128] agg PSUM.
                If tail_accumulates, tail issues further accumulating
                matmuls (incl. the stop); else the last scatter matmul
                carries stop=True.

                Gathers are emitted as SWDGE prepare+trigger pairs in
                batches of two groups (one prep per queue), so descriptor
                generation overlaps the inter-layer collective."""
                goffs = []
                off = 0
                for g in groups:
                    num = len(g) * q
                    goffs.append((g, off, off + num))
                    off += 2 * num
                for i0 in range(0, len(groups), 2):
                    batch = []
                    trig_q = []
                    for g, off_lo, off_hi in goffs[i0:i0 + 2]:
                        nw = len(g)
                        num = nw * q
                        cols = num // WIN_P
                        v_lo = gp.tile([128, gmaxc, 128], bf16, tag="glo",
                                       name=f"glo{layer}_{g[0]}")
                        v_hi = gp.tile([128, gmaxc, 128], bf16, tag="ghi",
                                       name=f"ghi{layer}_{g[0]}")
                        for v, t_src, ofs in ((v_lo, t_lo, off_lo),
                                              (v_hi, t_hi, off_hi)):
                            qn = qctr[0] % 4
                            nc.gpsimd.dma_gather(
                                v[:, 0:cols, :],
                                t_src, s_idx[:, ofs // 16: (ofs + num) // 16],
                                num, num, 128, elem_step=128,
                                single_packet=False,
                                queue_num=qn,
                            )
                            trig_q.append(qn)
                            qctr[0] += 1
                        batch.append((g, off_lo, off_hi, v_lo, v_hi))
                    for g, off, off2, v_lo, v_hi in batch:
                        consume_group(layer, g, off, off2, v_lo, v_hi,
                                      tail, tail_accumulates)

            def consume_group(layer, g, off, off2, v_lo, v_hi,
                              tail, tail_accumulates):
                for wi, w in enumerate(g):
                    ps = pagg.tile([128, 128], f32, tag="agg",
                                   name=f"agg{layer}_{w}")
                    n_mm = 2 * qc
                    mm = 0
                    for gbuf, base in ((v_lo, off), (v_hi, off2)):
                        # fused one-hot build: all qc chunks in one op
                        ccol0 = (base + wi * qc * WIN_P) // WIN_P
                        dmat = dp.tile([128, qc, 128], bf16, tag="dmat",
                                       name=f"d{layer}_{w}_{mm}")
                        dl3 = s_dlc[:, ccol0:ccol0 + qc].to_broadcast(
                            [128, qc, 128])
                        io3 = s_iota[:].rearrange("p (k n) -> p k n", k=qc)
                        nc.vector.tensor_tensor(
                            dmat[:], io3, dl3, alu.is_equal)
                        for k in range(qc):
                            is_last = mm == n_mm - 1
                            nc.tensor.matmul(
                                ps[:], gbuf[:, wi * qc + k, :],
                                dmat[:, k, :],
                                start=(mm == 0),
                                stop=(is_last and not tail_accumulates),
                                skip_group_check=True,
                            )
                            mm += 1
                    tail(w, ps)

            # ---- Layer 1: aggregate raw x (bf16), project after ----------
            def tail1(w, ps):
                ws = slice(w * WIN_P, (w + 1) * WIN_P)
                ag = ap_.tile([128, 128], bf16, tag="aggs", name=f"ag1_{w}")
                nc.vector.tensor_copy(ag[:], ps[:])
                ps_h = ph.tile([64, 128], f32, tag="psh", name=f"h1p_{w}")
                nc.tensor.matmul(ps_h[:], s_w1r[:], ag[:],
                                 start=True, stop=False)
                nc.tensor.matmul(ps_h[:], s_w1o[:], s_xT[:, ws],
                                 start=False, stop=True)
                nc.scalar.activation(h1T[:, ws], ps_h[:], act.Relu,
                                     bias=s_b1[:])
                # L2 table: p2 = h1 @ w2_rel, node-major directly
                ps_p = paux.tile([128, 128], f32, tag="aux", name=f"p2_{w}")
                nc.tensor.matmul(ps_p[:], h1T[:, ws], s_w2r[:],
                                 start=True, stop=True)
                st = sp.tile([128, 128], bf16, tag="stage", name=f"t2s_{w}")
                nc.vector.tensor_copy(st[:], ps_p[:])
                nc.sync.dma_start(tbl_in[1][ws, :], st[:])

            def allgather_split(i):
                # two half-collectives: A (first hsplit windows) can fire
                # during the tail drain; the next layer's lo gathers only
                # depend on A's output
                nc.gpsimd.collective_compute(
                    "AllGather", alu.bypass, replica_groups=rg,
                    ins=[tbl_in[i][0:hrows, :]], outs=[tbl_a[i][:]],
                )
                nc.gpsimd.collective_compute(
                    "AllGather", alu.bypass, replica_groups=rg,
                    ins=[tbl_in[i][hrows:shard, :]], outs=[tbl_b[i][:]],
                )

            agg_pass(0, xnode[0:half, :], xnode[half:npad, :], tail1, False)
            allgather_split(1)

            # ---- Layer 2: aggregate p2, root accumulates into agg --------
            def tail2(w, ps):
                ws = slice(w * WIN_P, (w + 1) * WIN_P)
                nc.tensor.matmul(ps[:], s_w2o[:], h1T[:, ws],
                                 start=False, stop=True, skip_group_check=True)
                nc.scalar.activation(h2T[:, ws], ps[:], act.Relu, bias=s_b2[:])
                # L3 table: h2 node-major via PE transpose
                ps_t = paux.tile([128, 128], bf16, tag="aux", name=f"t3p_{w}")
                nc.tensor.transpose(ps_t[:], h2T[:, ws], s_id[:])
                st = sp.tile([128, 128], bf16, tag="stage", name=f"t3s_{w}")
                nc.vector.tensor_copy(st[:], ps_t[:])
                nc.sync.dma_start(tbl_in[2][ws, :], st[:])

            agg_pass(1, tbl_a[1][:], tbl_b[1][:], tail2, True)
            allgather_split(2)

            # ---- Layer 3: aggregate h2, project after (192 = 128 + 64) ---
            def tail3(w, ps):
                ws = slice(w * WIN_P, (w + 1) * WIN_P)
                ag = ap_.tile([128, 128], bf16, tag="aggs", name=f"ag3_{w}")
                nc.vector.tensor_copy(ag[:], ps[:])
                ps_a = ph.tile([128, 128], f32, tag="psh", name=f"h3ap_{w}")
                nc.tensor.matmul(ps_a[:], s_w3r[:, 0:128], ag[:],
                                 start=True, stop=False)
                nc.tensor.matmul(ps_a[:], s_w3o[:, 0:128], h2T[:, ws],
                                 start=False, stop=True)
                nc.scalar.activation(h3aT[:, ws], ps_a[:], act.Relu,
                                     bias=s_b3a[:])
                ps_b = paux.tile([64, 128], f32, tag="aux", name=f"h3bp_{w}")
                nc.tensor.matmul(ps_b[:], s_w3r[:, 128:192], ag[:],
                                 start=True, stop=False)
                nc.tensor.matmul(ps_b[:], s_w3o[:, 128:192], h2T[:, ws],
                                 start=False, stop=True)
                nc.scalar.activation(h3bT[:, ws], ps_b[:], act.Relu,
                                     bias=s_b3b[:])
                # L4 table: p4 = h3 @ w4_rel node-major, duplicated [p4|p4]
                ps_p = ph.tile([128, 64], f32, tag="psh", name=f"p4_{w}")
                nc.tensor.matmul(ps_p[:], h3aT[:, ws], s_w4ra[:],
                                 start=True, stop=False)
                nc.tensor.matmul(ps_p[:], h3bT[:, ws], s_w4rb[:],
                                 start=False, stop=True)
                st = sp.tile([128, 128], bf16, tag="stage", name=f"t4s_{w}")
                nc.vector.tensor_copy(st[:, 0:64], ps_p[:])
                nc.vector.tensor_copy(st[:, 64:128], ps_p[:])
                nc.sync.dma_start(tbl_in[3][ws, :], st[:])

            agg_pass(2, tbl_a[2][:], tbl_b[2][:], tail3, False)
            allgather_split(3)

            # ---- Layer 4: aggregate [p4|p4]; rows 0:64 are the real agg --
            ps_g = ppl.tile([n_graphs, HID], f32)

            def tail4(w, ps):
                ws = slice(w * WIN_P, (w + 1) * WIN_P)
                nc.tensor.matmul(ps[:], s_w4oa[:], h3aT[:, ws],
                                 start=False, stop=False, skip_group_check=True)
                nc.tensor.matmul(ps[:], s_w4ob[:], h3bT[:, ws],
                                 start=False, stop=True, skip_group_check=True)
                nc.scalar.activation(h4T[:, ws], ps[0:64, :], act.Identity,
                                     bias=s_b4[:])
                # pooling partial: B.T @ h4_node
                ps_t = paux.tile([128, 64], bf16, tag="aux", name=f"h4n_{w}")
                nc.tensor.transpose(ps_t[:], h4T[:, ws], s_id[0:64, 0:64])
                h4n = sp.tile([128, 64], bf16, tag="h4n", name=f"h4s_{w}")
                nc.vector.tensor_copy(h4n[:], ps_t[:])
                bt = sp.tile([128, n_graphs], bf16, tag="btile", name=f"bt_{w}")
                nc.sync.dma_start(bt[:], bp[w * WIN_P:(w + 1) * WIN_P, :])
                nc.tensor.matmul(ps_g[:], bt[:], h4n[:],
                                 start=(w == 0), stop=(w == w_cnt - 1),
                                 skip_group_check=True)

            agg_pass(3, tbl_a[3][:], tbl_b[3][:], tail4, True)

            s_out = sp.tile([n_graphs, HID], f32, tag="out")
            nc.vector.tensor_copy(s_out[:], ps_g[:])
            nc.sync.dma_start(pooled[:], s_out[:])

    nc.compile()
    return nc


def _get_program(w_cnt, q, n_graphs, f_in):
    key = (w_cnt, q, n_graphs, f_in)
    if key not in _PROGRAM_CACHE:
        _PROGRAM_CACHE[key] = _build_program(w_cnt, q, n_graphs, f_in)
    return _PROGRAM_CACHE[key]


# --------------------------------------------------------------------------
# Execution
# --------------------------------------------------------------------------
def _in_maps(plan, inputs):
    maps = []
    qc = plan["q"] // WIN_P
    iota = np.tile(np.arange(128, dtype=np.float32), (128, qc)).astype(BF16)
    ident = np.eye(128, dtype=np.float32).astype(BF16)
    b3 = np.asarray(inputs["b3"], np.float32)
    w4o = np.asarray(inputs["w4_root"], np.float32)
    w4oa = np.zeros((128, 128), np.float32)
    w4oa[:, 0:64] = w4o[0:128]
    w4ob = np.zeros((64, 128), np.float32)
    w4ob[:, 0:64] = w4o[128:192]

    def b16(name):
        return np.asarray(inputs[name], np.float32).astype(BF16)

    for c in range(CORES):
        m = {
            "xnode": plan["xnode"],
            "xT": plan["x_t"][c],
            "idx": plan["idx_tile"][c],
            "dloc": plan["dloc_tile"][c],
            "bpool": plan["bpool"][c],
            "iota": iota,
            "ident": ident,
            "w1_rel": b16("w1_rel"),
            "w1_root": b16("w1_root"),
            "b1": np.asarray(inputs["b1"], np.float32).reshape(-1, 1),
            "w2_rel": b16("w2_rel"),
            "w2_root": b16("w2_root"),
            "b2": np.asarray(inputs["b2"], np.float32).reshape(-1, 1),
            "w3_rel": b16("w3_rel"),
            "w3_root": b16("w3_root"),
            "b3a": b3[:128].reshape(-1, 1),
            "b3b": b3[128:].reshape(-1, 1),
            "w4_rel_a": np.asarray(inputs["w4_rel"], np.float32)[0:128].astype(BF16),
            "w4_rel_b": np.asarray(inputs["w4_rel"], np.float32)[128:192].astype(BF16),
            "w4_root_a": w4oa.astype(BF16),
            "w4_root_b": w4ob.astype(BF16),
            "b4": np.asarray(inputs["b4"], np.float32).reshape(-1, 1),
        }
        maps.append(m)
    return maps


def _post(outs, inputs, n_graphs):
    total = np.zeros((n_graphs, HID), np.float32)
    for o in outs:
        total += o["pooled"]
    batch = np.asarray(inputs["batch"]).astype(np.int64)
    counts = np.bincount(batch, minlength=n_graphs).astype(np.float32)
    pooled = total / np.maximum(counts, 1.0)[:, None]
    hw = np.asarray(inputs["head_w"], np.float32)
    hb = np.asarray(inputs["head_b"], np.float32)
    return (pooled @ hw + hb).astype(np.float32)


def run(inputs, trace=False, sim=False, n_graphs=64):
    x = np.asarray(inputs["x"], np.float32)
    ei = np.asarray(inputs["edge_index"]).astype(np.int64)
    batch = np.asarray(inputs["batch"]).astype(np.int64)
    plan = _plan(x, ei[0], ei[1], batch, n_graphs)
    nc = _get_program(plan["w_cnt"], plan["q"], n_graphs, x.shape[1])
    maps = _in_maps(plan, inputs)

    if sim:
        from concourse.bass_interp import MultiCoreSim

        msim = MultiCoreSim(nc, num_cores=CORES)
        for c in range(CORES):
            for k, v in maps[c].items():
                msim.cores[c].tensor(k)[:] = v
        msim.simulate()
        outs = [
            {"pooled": np.array(msim.cores[c].tensor("pooled"))}
            for c in range(CORES)
        ]
        return _post(outs, inputs, n_graphs), None

    from concourse import bass_utils

    res = bass_utils.run_bass_kernel_spmd(
        nc, maps, core_ids=list(range(CORES)), trace=trace,
    )
    out = _post(res.results, inputs, n_graphs)
    return out, res


def kernel(**inputs) -> np.ndarray:
    out, _ = run(inputs)
    return out
